# revision 25
# baseline (speedup 1.0000x reference)
"""Trainium2 Bass kernel for nn_LinearInFieldChargesBlock (e3nn fully-connected
tensor product, forward only).

Math (per node n):
  out0[w] = 0.01*(C000 * sum_{u,v} x0[u] y0[v] w000[u,v,w]
                 + C110 * sum_{u,v,i} x1[u,i] y1[v,i] w110[u,v,w])
  out1[w,k] = 0.01*(C011 * sum_{u,v} x0[u] y1[v,k] w011[u,v,w]
                 + C101 * sum_{u,v} x1[u,k] y0[v] w101[u,v,w])
  out = concat([out0, out1.reshape(-1)]) with column 0 zeroed.

Kernel formulation:
  The bilinear form is decomposed into 160 rank-1 products q[f,n] =
  p[f,n] * yb[f,n], where p = W^T x (stage-1 PE matmuls contracting the
  512 node features, path constants folded into W) and yb[f,n] is the per-
  node y value of product f.  out[o,n] = sum_f R[f,o] q[f,n] (0/1 reduce
  matmuls, col 0 zeroed).

  The 160 product rows are split 128 ("A", psum tile pa) + 32 ("B", pb at
  partitions 32..63).  The broadcast tile yb is built once per DMA chunk
  with SBUF->SBUF gather DMAs (partition_broadcast access patterns), so
  the PE only runs 6 matmuls per 512-node sub-batch, packed into 3
  serial array slots via tile_position concurrency:
      [c0|c1|c2|pb]  ->  R_A(g-2)  ->  R_B(g-2)
  The reduce matmuls run at pipeline distance 2 so the in-order PE queue
  never waits on the scalar/DVE/gpsimd q-chain.

  All node data is staged bf16 host-side in transposed layout (features
  on partitions), halving HBM traffic and removing on-chip transposes.

Sharding: pure data-parallel across 8 cores along the node axis; the tiny
path-weight matrices are replicated.
"""

import sys

import numpy as np

try:
    import concourse  # noqa: F401
except ImportError:
    sys.path.insert(0, "/opt/trn_rl_repo")

N_NODES = 400000
N_CORES = 8
BATCH = 512            # nodes per PSUM sub-batch
SUB_PER_CHUNK = 2      # sub-batches per X-DMA chunk
CHUNK = BATCH * SUB_PER_CHUNK  # 1024 nodes per X-DMA chunk
N_CHUNKS = 49
PER_CORE = CHUNK * N_CHUNKS    # 50176 >= ceil(400000/8)
PADDED = PER_CORE * N_CORES

_S = 0.01
_CS000 = _S / 32.0
_CS110 = _S / (32.0 * np.sqrt(3.0))
_CS011 = _S / 32.0
_CS101 = _S / 32.0


def _bf16():
    import ml_dtypes

    return np.dtype(ml_dtypes.bfloat16)


def _build_mats(w000, w011, w101, w110):
    """Build the stage-1 weight blocks and the reduce matrices.

    Product-row layout, within-block index r = 4*v + w (v matches the
    gather-DMA partition stride, w is the 0-stride broadcast dim):
      pa rows  0..15 : t011 copy k=0  (chunk0)  factor y1[v,0]
      pa rows 16..31 : t011 copy k=1  (chunk0)  factor y1[v,1]
      pa rows 32..47 : t011 copy k=2  (chunk0)  factor y1[v,2]
      pa rows 48..63 : t000           (chunk0)  factor y0[v]
      pa rows 64..79 : t110 i=0       (chunk1)  factor y1[v,0]
      pa rows 80..95 : t101 k=0       (chunk1)  factor y0[v]
      pa rows 96..111: t110 i=1       (chunk2)  factor y1[v,1]
      pa rows112..127: t101 k=1       (chunk2)  factor y0[v]
      pb parts 32..47: t110 i=2       (chunk3)  factor y1[v,2]
      pb parts 48..63: t101 k=2       (chunk3)  factor y0[v]

    y feature order (natural pot_feat layout): b=v for y0[v]; b=4+3v+i
    for y1[v,i].  out columns: o=w for out0[w]; o=4+3w+k for out1[w,k];
    column 0 zeroed via R.
    """
    bf16 = _bf16()
    WA0 = np.zeros((128, 64), np.float32)   # chunk0 -> pa rows 0..63
    W1X = np.zeros((128, 32), np.float32)   # chunks 1..3 -> 32 rows each
    BA = np.zeros((16, 128), np.float32)
    RA = np.zeros((128, 16), np.float32)
    RB = np.zeros((32, 16), np.float32)
    for v in range(4):
        for w in range(4):
            r = 4 * v + w
            # pa blocks 0..2: t011 copies k=0,1,2
            for k in range(3):
                WA0[:, 16 * k + r] = _CS011 * w011[:, v, w]
                BA[4 + 3 * v + k, 16 * k + r] = 1.0
                RA[16 * k + r, 4 + 3 * w + k] = 1.0
            # pa block 3: t000
            WA0[:, 48 + r] = _CS000 * w000[:, v, w]
            BA[v, 48 + r] = 1.0
            if w > 0:
                RA[48 + r, w] = 1.0
            # shared chunk block (chunks 1..3 with i = chunk-1):
            #   rows 0..15: t110[i], rows 16..31: t101[k=i]
            W1X[:, r] = _CS110 * w110[:, v, w]
            W1X[:, 16 + r] = _CS101 * w101[:, v, w]
            # chunk1 (i=0) -> pa rows 64..95
            BA[4 + 3 * v + 0, 64 + r] = 1.0
            if w > 0:
                RA[64 + r, w] = 1.0
            BA[v, 80 + r] = 1.0
            RA[80 + r, 4 + 3 * w + 0] = 1.0
            # chunk2 (i=1) -> pa rows 96..111
            BA[4 + 3 * v + 1, 96 + r] = 1.0
            if w > 0:
                RA[96 + r, w] = 1.0
            BA[v, 112 + r] = 1.0
            RA[112 + r, 4 + 3 * w + 1] = 1.0
            # chunk3 (i=2) -> pb; factors match yba rows 32..63
            if w > 0:
                RB[r, w] = 1.0
            RB[16 + r, 4 + 3 * w + 2] = 1.0
    return (
        WA0.astype(bf16),
        W1X.astype(bf16),
        BA.astype(bf16),
        RA.astype(bf16),
        RB.astype(bf16),
    )


def _pack_inputs(node_feat, pot_feat):
    """Transpose + pad + bf16-cast the node data. Returns (xT, yT) with
    xT [128, 4, PADDED]: chunk 0 = x0 features, chunks 1..3 = x1[:, :, i-1].
    yT [16, PADDED]."""
    bf16 = _bf16()
    n = node_feat.shape[0]
    xT = np.zeros((128, 4, PADDED), dtype=bf16)
    xT[:, 0, :n] = np.asarray(node_feat[:, :128].T, dtype=bf16)
    x1 = node_feat[:, 128:].reshape(n, 128, 3)
    for c in range(3):
        xT[:, 1 + c, :n] = np.asarray(x1[:, :, c].T, dtype=bf16)
    yT = np.zeros((16, PADDED), dtype=bf16)
    yT[:, :n] = np.asarray(pot_feat.T, dtype=bf16)
    return xT, yT


def build_in_maps(node_feat, pot_feat, w000, w011, w101, w110):
    node_feat = np.asarray(node_feat, dtype=np.float32)
    pot_feat = np.asarray(pot_feat, dtype=np.float32)
    WA0, W1X, BA, RA, RB = _build_mats(
        np.asarray(w000, np.float32),
        np.asarray(w011, np.float32),
        np.asarray(w101, np.float32),
        np.asarray(w110, np.float32),
    )
    xT, yT = _pack_inputs(node_feat, pot_feat)
    in_maps = []
    for i in range(N_CORES):
        sl = slice(i * PER_CORE, (i + 1) * PER_CORE)
        in_maps.append(
            {
                "xt": np.ascontiguousarray(xT[:, :, sl]),
                "yt": np.ascontiguousarray(yT[:, sl]),
                "wa0": WA0,
                "w1x": W1X,
                "ba": BA,
                "ra": RA,
                "rb": RB,
            }
        )
    return in_maps


_CACHE = {}


def build_kernel(n_nodes=PER_CORE):
    """Build + compile the per-core Bass program (n_nodes multiple of CHUNK)."""
    if n_nodes in _CACHE:
        return _CACHE[n_nodes]

    import concourse.bacc as bacc
    import concourse.tile as tile
    from concourse import mybir

    f32 = mybir.dt.float32
    bf = mybir.dt.bfloat16

    assert n_nodes % CHUNK == 0
    n_chunks = n_nodes // CHUNK
    n_batches = n_chunks * SUB_PER_CHUNK

    nc = bacc.Bacc(None, target_bir_lowering=False)
    xtd = nc.dram_tensor("xt", [128, 4, n_nodes], bf, kind="ExternalInput")
    ytd = nc.dram_tensor("yt", [16, n_nodes], bf, kind="ExternalInput")
    wa0d = nc.dram_tensor("wa0", [128, 64], bf, kind="ExternalInput")
    w1xd = nc.dram_tensor("w1x", [128, 32], bf, kind="ExternalInput")
    bad = nc.dram_tensor("ba", [16, 128], bf, kind="ExternalInput")
    rad = nc.dram_tensor("ra", [128, 16], bf, kind="ExternalInput")
    rbd = nc.dram_tensor("rb", [32, 16], bf, kind="ExternalInput")
    outd = nc.dram_tensor("out", [16, n_nodes], bf, kind="ExternalOutput")

    with tile.TileContext(nc) as tc:
        with (
            tc.tile_pool(name="consts", bufs=1) as consts,
            tc.tile_pool(name="xin", bufs=6) as xin,
            tc.tile_pool(name="yin", bufs=1) as yin,
            tc.tile_pool(name="stg", bufs=2) as stgp,
            tc.tile_pool(name="ybs", bufs=3) as ybsp,
            tc.tile_pool(name="pas", bufs=3) as pasp,
            tc.tile_pool(name="qa", bufs=3) as qap,
            tc.tile_pool(name="qb", bufs=3) as qbp,
            tc.tile_pool(name="pa", bufs=2, space="PSUM") as pap,
            tc.tile_pool(name="pb", bufs=2, space="PSUM") as pbp,
            tc.tile_pool(name="yba", bufs=2, space="PSUM") as ybap,
            tc.tile_pool(name="otq", bufs=1, space="PSUM") as otqp,
        ):
            wa0 = consts.tile([128, 64], bf, tag="wa0")
            nc.sync.dma_start(out=wa0[:], in_=wa0d[:])
            w1x = consts.tile([128, 32], bf, tag="w1x")
            nc.sync.dma_start(out=w1x[:], in_=w1xd[:])
            ba = consts.tile([16, 128], bf, tag="ba")
            nc.sync.dma_start(out=ba[:], in_=bad[:])
            ra = consts.tile([128, 16], bf, tag="ra")
            nc.sync.dma_start(out=ra[:], in_=rad[:])
            rbt = consts.tile([64, 16], bf, tag="rb")
            nc.sync.dma_start(out=rbt[32:64, :], in_=rbd[:])

            # all of Y stays resident: no per-chunk Y dependency at all
            Y = yin.tile([16, n_nodes], bf, tag="y")
            nc.sync.dma_start(out=Y[:], in_=ytd[:])

            X = None
            otq_cur = [None]  # current 2-batch group tile
            pending = []  # reduce states, drained at pipeline distance 2

            def emit_reduce(state):
                """Reduce one batch into its bank of a 2-bank group tile;
                after both batches of the group, one cast + one contiguous
                DMA moves the group out (halves the DVE/DMA op count).
                Each batch's R_A/R_B accumulation stays within ONE bank, so
                concurrent matmuls of adjacent batches never share a bank."""
                qa_, qb_, gg = state
                j = gg % 2
                if j == 0:
                    otq_cur[0] = otqp.tile(
                        [16, 2 * BATCH], f32, tag="otq", name=f"otq_{gg}"
                    )
                otq = otq_cur[0]
                sl = otq[:, j * BATCH : (j + 1) * BATCH]
                nc.tensor.matmul(sl, ra[:], qa_[:], start=True, stop=False)
                nc.tensor.matmul(
                    sl, rbt[32:64, :], qb_[32:64, :], start=False, stop=True
                )
                if j == 1:
                    stage = stgp.tile([16, 2 * BATCH], bf, tag="stg")
                    nc.vector.tensor_copy(stage[:], otq[:])
                    # out-DMA on the ACT HWDGE ring: the Sync ring carries
                    # the input prefetch and must never head-of-line block
                    # on a DMA that waits for compute.
                    c0 = (gg - 1) * BATCH
                    nc.scalar.dma_start(
                        out=outd[:, c0 : c0 + 2 * BATCH], in_=stage[:]
                    )

            for g in range(n_batches):
                ch, sb = divmod(g, SUB_PER_CHUNK)
                j0 = ch * CHUNK
                if sb == 0:
                    X = xin.tile([128, 4 * CHUNK], bf, tag="x")
                    nc.sync.dma_start(
                        out=X[:].rearrange("p (c m) -> p c m", c=4),
                        in_=xtd[:, :, j0 : j0 + CHUNK],
                    )

                m0 = sb * BATCH
                # broadcast y into the 128 A-rows (selector matmul)
                yba = ybap.tile([128, BATCH], f32, tag="yba")
                nc.tensor.matmul(
                    yba[:], ba[:],
                    Y[:, g * BATCH : (g + 1) * BATCH],
                    start=True, stop=True,
                )
                ybs = ybsp.tile([128, BATCH], bf, tag="ybs")
                nc.scalar.copy(out=ybs[:], in_=yba[:])
                # stage 1: contract the 512 node features.
                # PE slot structure per batch (disjoint array cells overlap):
                #   [c0|c1|c2|pb] -> [R_A(g-2)|R_B(g-3)] -> Yb_A
                pa = pap.tile([128, BATCH], f32, tag="pa")
                nc.tensor.matmul(
                    pa[0:64, :],
                    wa0[:],
                    X[:, 0 * CHUNK + m0 : 0 * CHUNK + m0 + BATCH],
                    start=True,
                    stop=True,
                )
                nc.tensor.matmul(
                    pa[64:96, :],
                    w1x[:],
                    X[:, 1 * CHUNK + m0 : 1 * CHUNK + m0 + BATCH],
                    start=True,
                    stop=True,
                )
                nc.tensor.matmul(
                    pa[96:128, :],
                    w1x[:],
                    X[:, 2 * CHUNK + m0 : 2 * CHUNK + m0 + BATCH],
                    start=True,
                    stop=True,
                    tile_position=(0, 96),
                )
                pb = pbp.tile([64, BATCH], f32, tag="pb")
                nc.tensor.matmul(
                    pb[32:64, :],
                    w1x[:],
                    X[:, 3 * CHUNK + m0 : 3 * CHUNK + m0 + BATCH],
                    start=True,
                    stop=True,
                )
                pas = pasp.tile([128, BATCH], bf, tag="pas")
                nc.scalar.copy(out=pas[:], in_=pa[:])
                # reduce of batch g-2 (q-chain has 2 batches of slack)
                if len(pending) >= 2:
                    emit_reduce(pending.pop(0))
                # q = p * yb (qa on gpsimd frees the DVE for qb+output)
                qa = qap.tile([128, BATCH], bf, tag="qa")
                nc.gpsimd.tensor_mul(qa[:], pas[:], ybs[:])
                qb = qbp.tile([64, BATCH], bf, tag="qb")
                nc.vector.tensor_mul(qb[32:64, :], pb[32:64, :], ybs[32:64, :])
                pending.append((qa, qb, g))
            for state in pending:
                emit_reduce(state)
    nc.compile()
    _CACHE[n_nodes] = nc
    return nc


def kernel(node_feat, pot_feat, w000, w011, w101, w110, **extra_kwargs):
    from concourse.bass_utils import run_bass_kernel_spmd

    n = np.asarray(node_feat).shape[0]
    in_maps = build_in_maps(node_feat, pot_feat, w000, w011, w101, w110)
    nc = build_kernel(PER_CORE)
    res = run_bass_kernel_spmd(nc, in_maps, core_ids=list(range(N_CORES)))
    outT = np.concatenate(
        [np.asarray(res.results[i]["out"]) for i in range(N_CORES)], axis=1
    )
    out = outT[:, :n].T.astype(np.float32)
    return np.ascontiguousarray(out)


# revision 26
# speedup vs baseline: 1.0578x; 1.0578x over previous
"""Trainium2 Bass kernel for nn_LinearInFieldChargesBlock (e3nn fully-connected
tensor product, forward only).

Math (per node n):
  out0[w] = 0.01*(C000 * sum_{u,v} x0[u] y0[v] w000[u,v,w]
                 + C110 * sum_{u,v,i} x1[u,i] y1[v,i] w110[u,v,w])
  out1[w,k] = 0.01*(C011 * sum_{u,v} x0[u] y1[v,k] w011[u,v,w]
                 + C101 * sum_{u,v} x1[u,k] y0[v] w101[u,v,w])
  out = concat([out0, out1.reshape(-1)]) with column 0 zeroed.

Kernel formulation:
  The bilinear form is decomposed into 160 rank-1 products q[f,n] =
  p[f,n] * yb[f,n], where p = W^T x (stage-1 PE matmuls contracting the
  512 node features, path constants folded into W) and yb[f,n] is the per-
  node y value of product f.  out[o,n] = sum_f R[f,o] q[f,n] (0/1 reduce
  matmuls, col 0 zeroed).

  The 160 product rows are split 128 ("A", psum tile pa) + 32 ("B", pb at
  partitions 32..63).  The broadcast tile yb is built once per DMA chunk
  with SBUF->SBUF gather DMAs (partition_broadcast access patterns), so
  the PE only runs 6 matmuls per 512-node sub-batch, packed into 3
  serial array slots via tile_position concurrency:
      [c0|c1|c2|pb]  ->  R_A(g-2)  ->  R_B(g-2)
  The reduce matmuls run at pipeline distance 2 so the in-order PE queue
  never waits on the scalar/DVE/gpsimd q-chain.

  All node data is staged bf16 host-side in transposed layout (features
  on partitions), halving HBM traffic and removing on-chip transposes.

Sharding: pure data-parallel across 8 cores along the node axis; the tiny
path-weight matrices are replicated.
"""

import sys

import numpy as np

try:
    import concourse  # noqa: F401
except ImportError:
    sys.path.insert(0, "/opt/trn_rl_repo")

N_NODES = 400000
N_CORES = 8
BATCH = 512            # nodes per PSUM sub-batch
SUB_PER_CHUNK = 2      # sub-batches per X-DMA chunk
CHUNK = BATCH * SUB_PER_CHUNK  # 1024 nodes per X-DMA chunk
N_CHUNKS = 49
PER_CORE = CHUNK * N_CHUNKS    # 50176 >= ceil(400000/8)
PADDED = PER_CORE * N_CORES

_S = 0.01
_CS000 = _S / 32.0
_CS110 = _S / (32.0 * np.sqrt(3.0))
_CS011 = _S / 32.0
_CS101 = _S / 32.0


def _bf16():
    import ml_dtypes

    return np.dtype(ml_dtypes.bfloat16)


def _build_mats(w000, w011, w101, w110):
    """Build the stage-1 weight blocks and the reduce matrices.

    Product-row layout, within-block index r = 4*v + w (v matches the
    gather-DMA partition stride, w is the 0-stride broadcast dim):
      pa rows  0..15 : t011 copy k=0  (chunk0)  factor y1[v,0]
      pa rows 16..31 : t011 copy k=1  (chunk0)  factor y1[v,1]
      pa rows 32..47 : t011 copy k=2  (chunk0)  factor y1[v,2]
      pa rows 48..63 : t000           (chunk0)  factor y0[v]
      pa rows 64..79 : t110 i=0       (chunk1)  factor y1[v,0]
      pa rows 80..95 : t101 k=0       (chunk1)  factor y0[v]
      pa rows 96..111: t110 i=1       (chunk2)  factor y1[v,1]
      pa rows112..127: t101 k=1       (chunk2)  factor y0[v]
      pb parts 32..47: t110 i=2       (chunk3)  factor y1[v,2]
      pb parts 48..63: t101 k=2       (chunk3)  factor y0[v]

    y feature order (natural pot_feat layout): b=v for y0[v]; b=4+3v+i
    for y1[v,i].  out columns: o=w for out0[w]; o=4+3w+k for out1[w,k];
    column 0 zeroed via R.
    """
    bf16 = _bf16()
    WA0 = np.zeros((128, 64), np.float32)   # chunk0 -> pa rows 0..63
    W1X = np.zeros((128, 32), np.float32)   # chunks 1..3 -> 32 rows each
    BA = np.zeros((16, 128), np.float32)
    RA = np.zeros((128, 16), np.float32)
    RB = np.zeros((32, 16), np.float32)
    for v in range(4):
        for w in range(4):
            r = 4 * v + w
            # pa blocks 0..2: t011 copies k=0,1,2
            for k in range(3):
                WA0[:, 16 * k + r] = _CS011 * w011[:, v, w]
                BA[4 + 3 * v + k, 16 * k + r] = 1.0
                RA[16 * k + r, 4 + 3 * w + k] = 1.0
            # pa block 3: t000
            WA0[:, 48 + r] = _CS000 * w000[:, v, w]
            BA[v, 48 + r] = 1.0
            if w > 0:
                RA[48 + r, w] = 1.0
            # shared chunk block (chunks 1..3 with i = chunk-1):
            #   rows 0..15: t110[i], rows 16..31: t101[k=i]
            W1X[:, r] = _CS110 * w110[:, v, w]
            W1X[:, 16 + r] = _CS101 * w101[:, v, w]
            # chunk1 (i=0) -> pa rows 64..95
            BA[4 + 3 * v + 0, 64 + r] = 1.0
            if w > 0:
                RA[64 + r, w] = 1.0
            BA[v, 80 + r] = 1.0
            RA[80 + r, 4 + 3 * w + 0] = 1.0
            # chunk2 (i=1) -> pa rows 96..111
            BA[4 + 3 * v + 1, 96 + r] = 1.0
            if w > 0:
                RA[96 + r, w] = 1.0
            BA[v, 112 + r] = 1.0
            RA[112 + r, 4 + 3 * w + 1] = 1.0
            # chunk3 (i=2) -> pb; factors match yba rows 32..63
            if w > 0:
                RB[r, w] = 1.0
            RB[16 + r, 4 + 3 * w + 2] = 1.0
    return (
        WA0.astype(bf16),
        W1X.astype(bf16),
        BA.astype(bf16),
        RA.astype(bf16),
        RB.astype(bf16),
    )


def _pack_inputs(node_feat, pot_feat):
    """Transpose + pad + bf16-cast the node data. Returns (xT, yT) with
    xT [128, 4, PADDED]: chunk 0 = x0 features, chunks 1..3 = x1[:, :, i-1].
    yT [16, PADDED]."""
    bf16 = _bf16()
    n = node_feat.shape[0]
    xT = np.zeros((128, 4, PADDED), dtype=bf16)
    xT[:, 0, :n] = np.asarray(node_feat[:, :128].T, dtype=bf16)
    x1 = node_feat[:, 128:].reshape(n, 128, 3)
    for c in range(3):
        xT[:, 1 + c, :n] = np.asarray(x1[:, :, c].T, dtype=bf16)
    yT = np.zeros((16, PADDED), dtype=bf16)
    yT[:, :n] = np.asarray(pot_feat.T, dtype=bf16)
    return xT, yT


def build_in_maps(node_feat, pot_feat, w000, w011, w101, w110):
    node_feat = np.asarray(node_feat, dtype=np.float32)
    pot_feat = np.asarray(pot_feat, dtype=np.float32)
    WA0, W1X, BA, RA, RB = _build_mats(
        np.asarray(w000, np.float32),
        np.asarray(w011, np.float32),
        np.asarray(w101, np.float32),
        np.asarray(w110, np.float32),
    )
    xT, yT = _pack_inputs(node_feat, pot_feat)
    in_maps = []
    for i in range(N_CORES):
        sl = slice(i * PER_CORE, (i + 1) * PER_CORE)
        in_maps.append(
            {
                "xt": np.ascontiguousarray(xT[:, :, sl]),
                "yt": np.ascontiguousarray(yT[:, sl]),
                "wa0": WA0,
                "w1x": W1X,
                "ba": BA,
                "ra": RA,
                "rb": RB,
            }
        )
    return in_maps


_CACHE = {}


def build_kernel(n_nodes=PER_CORE):
    """Build + compile the per-core Bass program (n_nodes multiple of CHUNK)."""
    if n_nodes in _CACHE:
        return _CACHE[n_nodes]

    import concourse.bacc as bacc
    import concourse.tile as tile
    from concourse import mybir
    from concourse.tile import add_dep_helper

    f32 = mybir.dt.float32
    bf = mybir.dt.bfloat16

    assert n_nodes % CHUNK == 0
    n_chunks = n_nodes // CHUNK
    n_batches = n_chunks * SUB_PER_CHUNK

    nc = bacc.Bacc(None, target_bir_lowering=False)
    xtd = nc.dram_tensor("xt", [128, 4, n_nodes], bf, kind="ExternalInput")
    ytd = nc.dram_tensor("yt", [16, n_nodes], bf, kind="ExternalInput")
    wa0d = nc.dram_tensor("wa0", [128, 64], bf, kind="ExternalInput")
    w1xd = nc.dram_tensor("w1x", [128, 32], bf, kind="ExternalInput")
    bad = nc.dram_tensor("ba", [16, 128], bf, kind="ExternalInput")
    rad = nc.dram_tensor("ra", [128, 16], bf, kind="ExternalInput")
    rbd = nc.dram_tensor("rb", [32, 16], bf, kind="ExternalInput")
    outd = nc.dram_tensor("out", [16, n_nodes], bf, kind="ExternalOutput")

    with tile.TileContext(nc) as tc:
        with (
            tc.tile_pool(name="consts", bufs=1) as consts,
            tc.tile_pool(name="xin", bufs=6) as xin,
            tc.tile_pool(name="yin", bufs=1) as yin,
            tc.tile_pool(name="stg", bufs=2) as stgp,
            tc.tile_pool(name="ybs", bufs=3) as ybsp,
            tc.tile_pool(name="pas", bufs=3) as pasp,
            tc.tile_pool(name="qa", bufs=3) as qap,
            tc.tile_pool(name="qb", bufs=3) as qbp,
            tc.tile_pool(name="pa", bufs=2, space="PSUM") as pap,
            tc.tile_pool(name="pb", bufs=2, space="PSUM") as pbp,
            tc.tile_pool(name="yba", bufs=2, space="PSUM") as ybap,
            tc.tile_pool(name="otq", bufs=1, space="PSUM") as otqp,
        ):
            wa0 = consts.tile([128, 64], bf, tag="wa0")
            nc.sync.dma_start(out=wa0[:], in_=wa0d[:])
            w1x = consts.tile([128, 32], bf, tag="w1x")
            nc.sync.dma_start(out=w1x[:], in_=w1xd[:])
            ba = consts.tile([16, 128], bf, tag="ba")
            nc.sync.dma_start(out=ba[:], in_=bad[:])
            ra = consts.tile([128, 16], bf, tag="ra")
            nc.sync.dma_start(out=ra[:], in_=rad[:])
            rbt = consts.tile([64, 16], bf, tag="rb")
            nc.sync.dma_start(out=rbt[32:64, :], in_=rbd[:])

            # all of Y stays resident: no per-chunk Y dependency at all
            Y = yin.tile([16, n_nodes], bf, tag="y")
            nc.sync.dma_start(out=Y[:], in_=ytd[:])

            X = None
            otq_cur = [None]  # current 2-batch group tile
            pending = []  # reduce states, drained at pipeline distance 2
            hist = {}     # g -> (qa_i, qb_i, ybs_i) instruction handles

            def emit_reduce(state):
                """Reduce one batch into its bank of a 2-bank group tile;
                after both batches of the group, one cast + one contiguous
                DMA moves the group out (halves the DVE/DMA op count).
                Each batch's R_A/R_B accumulation stays within ONE bank, so
                concurrent matmuls of adjacent batches never share a bank."""
                qa_, qb_, gg, after = state
                j = gg % 2
                if j == 0:
                    otq_cur[0] = otqp.tile(
                        [16, 2 * BATCH], f32, tag="otq", name=f"otq_{gg}"
                    )
                otq = otq_cur[0]
                sl = otq[:, j * BATCH : (j + 1) * BATCH]
                ra_i = nc.tensor.matmul(sl, ra[:], qa_[:], start=True, stop=False)
                if after is not None:
                    add_dep_helper(ra_i.ins, after.ins, sync=False,
                                   reason="pin PE order: reduce after stage1")
                rb_i = nc.tensor.matmul(
                    sl, rbt[32:64, :], qb_[32:64, :], start=False, stop=True
                )
                emit_reduce.last_rb = rb_i
                if j == 1:
                    stage = stgp.tile([16, 2 * BATCH], bf, tag="stg")
                    nc.vector.tensor_copy(stage[:], otq[:])
                    # out-DMA on the ACT HWDGE ring: the Sync ring carries
                    # the input prefetch and must never head-of-line block
                    # on a DMA that waits for compute.
                    c0 = (gg - 1) * BATCH
                    nc.scalar.dma_start(
                        out=outd[:, c0 : c0 + 2 * BATCH], in_=stage[:]
                    )

            for g in range(n_batches):
                ch, sb = divmod(g, SUB_PER_CHUNK)
                j0 = ch * CHUNK
                if sb == 0:
                    X = xin.tile([128, 4 * CHUNK], bf, tag="x")
                    nc.sync.dma_start(
                        out=X[:].rearrange("p (c m) -> p c m", c=4),
                        in_=xtd[:, :, j0 : j0 + CHUNK],
                    )

                m0 = sb * BATCH
                # stage 1: contract the 512 node features.
                # PE slot structure per batch (disjoint array cells overlap):
                #   [c0|c1|c2|pb] -> [R_A(g-2)|R_B(g-3)] -> Yb_A
                pa = pap.tile([128, BATCH], f32, tag="pa")
                c0_i = nc.tensor.matmul(
                    pa[0:64, :],
                    wa0[:],
                    X[:, 0 * CHUNK + m0 : 0 * CHUNK + m0 + BATCH],
                    start=True,
                    stop=True,
                )
                # hoist the cross-engine waits of the whole batch onto c0:
                # the later matmuls then carry no semaphore waits and can
                # issue back-to-back / concurrently in the PE array.
                if g - 2 in hist:
                    for dep in hist.pop(g - 2):
                        add_dep_helper(
                            c0_i.ins, dep.ins, sync=True,
                            reason="hoist batch waits onto c0",
                        )
                c1_i = nc.tensor.matmul(
                    pa[64:96, :],
                    w1x[:],
                    X[:, 1 * CHUNK + m0 : 1 * CHUNK + m0 + BATCH],
                    start=True,
                    stop=True,
                )
                add_dep_helper(c1_i.ins, c0_i.ins, sync=False, reason="order")
                c2_i = nc.tensor.matmul(
                    pa[96:128, :],
                    w1x[:],
                    X[:, 2 * CHUNK + m0 : 2 * CHUNK + m0 + BATCH],
                    start=True,
                    stop=True,
                    tile_position=(0, 96),
                )
                add_dep_helper(c2_i.ins, c1_i.ins, sync=False, reason="order")
                pb = pbp.tile([64, BATCH], f32, tag="pb")
                pb_i = nc.tensor.matmul(
                    pb[32:64, :],
                    w1x[:],
                    X[:, 3 * CHUNK + m0 : 3 * CHUNK + m0 + BATCH],
                    start=True,
                    stop=True,
                )
                add_dep_helper(pb_i.ins, c2_i.ins, sync=False, reason="order")
                pas = pasp.tile([128, BATCH], bf, tag="pas")
                nc.scalar.copy(out=pas[:], in_=pa[:])
                # reduce of batch g-2 (q-chain has 2 batches of slack)
                if len(pending) >= 2:
                    emit_reduce(pending.pop(0) + (pb_i,))
                # broadcast y into the 128 A-rows (selector matmul), after
                # the reduce pair in the PE queue (disjoint cells with R_B)
                yba = ybap.tile([128, BATCH], f32, tag="yba")
                yb_i = nc.tensor.matmul(
                    yba[:], ba[:],
                    Y[:, g * BATCH : (g + 1) * BATCH],
                    start=True, stop=True,
                )
                last_rb = getattr(emit_reduce, "last_rb", None)
                if last_rb is not None:
                    add_dep_helper(yb_i.ins, last_rb.ins, sync=False,
                                   reason="order")
                ybs = ybsp.tile([128, BATCH], bf, tag="ybs")
                ybs_i = nc.scalar.copy(out=ybs[:], in_=yba[:])
                # q = p * yb (qa on gpsimd frees the DVE for qb+output)
                qa = qap.tile([128, BATCH], bf, tag="qa")
                qa_i = nc.gpsimd.tensor_mul(qa[:], pas[:], ybs[:])
                qb = qbp.tile([64, BATCH], bf, tag="qb")
                qb_i = nc.vector.tensor_mul(
                    qb[32:64, :], pb[32:64, :], ybs[32:64, :]
                )
                hist[g] = (qa_i, qb_i, ybs_i)
                pending.append((qa, qb, g))
            for state in pending:
                emit_reduce(state + (None,))
    nc.compile()
    _CACHE[n_nodes] = nc
    return nc


def kernel(node_feat, pot_feat, w000, w011, w101, w110, **extra_kwargs):
    from concourse.bass_utils import run_bass_kernel_spmd

    n = np.asarray(node_feat).shape[0]
    in_maps = build_in_maps(node_feat, pot_feat, w000, w011, w101, w110)
    nc = build_kernel(PER_CORE)
    res = run_bass_kernel_spmd(nc, in_maps, core_ids=list(range(N_CORES)))
    outT = np.concatenate(
        [np.asarray(res.results[i]["out"]) for i in range(N_CORES)], axis=1
    )
    out = outT[:, :n].T.astype(np.float32)
    return np.ascontiguousarray(out)
